# revision 1
# baseline (speedup 1.0000x reference)
"""MetaOptNet episode kernel for 8x Trainium2 NeuronCores.

Math (from the reference nn.Module):
    x: [15025, 4096] = 5 classes x (5 support + 3000 query) rows.
    K = support @ support.T  (25x25)
    qp = interior-point solve of a tiny 125-var SVM dual (15 fixed iterations)
    logits = (query @ support.T) @ qp        -> [15000, 5]

Split of work:
  - The QP solve is a tiny serial 125-variable problem (15 iterations of a
    150x150 linear solve); it is replicated on the host in float32, exactly
    mirroring the reference algorithm step by step.
  - The memory-bound bulk (reading 235 MB of query rows and contracting
    them against support) runs on the 8 NeuronCores, data-parallel over
    query rows: each core streams its 1875-query shard (padded to NQ_PAD)
    and computes logits.T for that shard. qp is folded into
    W = sup.T @ qp on the host, so the device is a single streaming
    accumulate-matmul: logits.T = sum_k W_k.T @ x_k.

Device-side layout trick: the query shard is pre-transposed on the host to
feature-major [32, 128, PL, NQ_PAD], so the contraction dim (d=4096) lands
on SBUF partitions naturally and the kernel needs zero on-chip transposes:
it is a pure streaming accumulate-matmul at the HBM roofline.

Precision modes (MK_STREAM_DT):
  - "hilo" (default): x and support are split on the host into bf16 hi/lo
    pairs (same total bytes as fp32). The device accumulates the three
    significant cross products hi*hi + hi*lo + lo*hi in fp32 PSUM, giving
    ~1e-5 relative error at full bf16 matmul throughput.
  - "f32": fp32 stream; matmuls run as float32r (TF32-like fast path,
    ~2e-4 relative error) or true fp32 with MK_MM_DT=f32.
  - "bf16": plain bf16 stream (half the DMA bytes, ~1.5e-3 rel error).
"""

import os

import numpy as np

# ---------------------------------------------------------------- constants
N_WAY = 5
N_SUPPORT = 5
N_QUERY = 3000
D = 4096
C_REG = 0.1
MAX_ITER = 15
SIGMA = 0.1

N_CORES = 8
NS = N_WAY * N_SUPPORT          # 25 support rows
NQ_TOT = N_WAY * N_QUERY        # 15000 query rows
NQ_SHARD = NQ_TOT // N_CORES    # 1875 per core
KCH = D // 128                  # 32 contraction chunks of 128
NS_PAD = 32                     # support rows padded (zero) to 32
NW_PAD = 8                      # classes padded (zero) to 8

# knobs for experiments (defaults are the shipping config)
STREAM_DT = os.environ.get("MK_STREAM_DT", "hilo")  # "hilo" | "f32" | "bf16"
MM_DT = os.environ.get("MK_MM_DT", "f32r")          # for "f32": "f32r" | "f32"
# f32r matmuls require an even moving (free) dim (1880/470); bf16 doesn't
_dq = ("1880", "470") if STREAM_DT == "f32" else ("1876", "469")
NQ_PAD = int(os.environ.get("MK_NQ_PAD", _dq[0]))   # padded per-core queries
QBLK = int(os.environ.get("MK_QBLK", _dq[1]))       # query block per matmul
SLAB = int(os.environ.get("MK_SLAB", "1"))          # k-chunks per DMA
SBUFS = int(os.environ.get("MK_BUFS", "8"))         # stream pool buffers
NQB = NQ_PAD // QBLK            # query blocks (psum banks)
PL = 2 if STREAM_DT == "hilo" else 1                # precision planes
assert NQ_PAD % QBLK == 0 and QBLK <= 512 and NQ_PAD >= NQ_SHARD


# ------------------------------------------------------------ host QP solve
def _qp_solve_host(K):
    """Mirror of reference._qp_solve for this problem's fixed G/e/C/h/A/b.

    C is the identity and b is zero, so C-products are elided (exact in
    fp32).  All arithmetic in float32 to track the reference's rounding.
    """
    dt = np.float32
    n = NS * N_WAY                                    # 125
    m, p = n, NS                                      # 125, 25
    G = np.kron(K, np.eye(N_WAY, dtype=dt)).astype(dt) + np.eye(n, dtype=dt)
    y = np.repeat(np.arange(N_WAY), N_SUPPORT)
    y1 = np.eye(N_WAY, dtype=dt)[y].reshape(-1)       # [125] one-hot flat
    e = -y1
    h = (dt(C_REG) * y1).astype(dt)
    A = np.kron(np.eye(NS, dtype=dt), np.ones((1, N_WAY), dtype=dt)).astype(dt)
    sigma = dt(SIGMA)

    z = np.zeros(n, dt)
    s = np.ones(m, dt)
    lam = np.ones(m, dt)
    nu = np.zeros(p, dt)

    for _ in range(MAX_ITER):
        r_dual = G @ z + e + lam + A.T @ nu
        r_pin = z + s - h
        r_peq = A @ z
        mu = np.dot(s, lam) / dt(m)
        r_cent = s * lam - sigma * mu
        w = lam / s
        M = G + np.diag(w).astype(dt)
        rhs_z = -(r_dual + (-r_cent + lam * r_pin) / s)
        KKT = np.block([[M, A.T], [A, np.zeros((p, p), dt)]]).astype(dt)
        sol = np.linalg.solve(KKT, np.concatenate([rhs_z, -r_peq]))
        dz, dnu = sol[:n], sol[n:]
        ds = -r_pin - dz
        dlam = (-r_cent - lam * ds) / s
        with np.errstate(divide="ignore", invalid="ignore"):
            a_s = np.min(np.where(ds < 0, -s / ds, np.inf)).astype(dt)
            a_l = np.min(np.where(dlam < 0, -lam / dlam, np.inf)).astype(dt)
        alpha = np.minimum(dt(1.0), dt(0.99) * np.minimum(a_s, a_l))
        z = z + alpha * dz
        s = s + alpha * ds
        lam = lam + alpha * dlam
        nu = nu + alpha * dnu

    return z.reshape(NS, N_WAY)                       # [25, 5]


# ------------------------------------------------------------- bass builder
_BUILD_CACHE = {}


def _np_stream_dtype():
    if STREAM_DT in ("bf16", "hilo"):
        import ml_dtypes

        return np.dtype(ml_dtypes.bfloat16)
    return np.dtype(np.float32)


def _build_bass():
    key = (STREAM_DT, MM_DT, NQ_PAD, QBLK, SLAB, SBUFS)
    if key in _BUILD_CACHE:
        return _BUILD_CACHE[key]

    import concourse.bacc as bacc
    import concourse.mybir as mybir
    import concourse.tile as tile
    from concourse.bass import ts

    if STREAM_DT in ("bf16", "hilo"):
        sdt = mybir.dt.bfloat16
    elif MM_DT == "f32r":
        # fp32 bytes, declared float32r end-to-end so the verifier sees
        # f32r provenance into the fast-path matmuls
        sdt = mybir.dt.float32r
    else:
        sdt = mybir.dt.float32
    f32 = mybir.dt.float32

    nc = bacc.Bacc("TRN2", target_bir_lowering=False, debug=False)
    xt = nc.dram_tensor("xt", [KCH, 128, PL, NQ_PAD], sdt, kind="ExternalInput")
    # W = sup.T @ qp, folded on the host: [128, KCH, PL, NW_PAD]
    whl = nc.dram_tensor("whl", [128, KCH, PL, NW_PAD], sdt, kind="ExternalInput")
    outT = nc.dram_tensor("outT", [NW_PAD, NQ_PAD], f32, kind="ExternalOutput")

    # (x_plane, w_plane) cross terms; lo*lo is ~2^-19 relative, dropped
    combos = [(0, 0)] if PL == 1 else [(0, 0), (1, 0), (0, 1)]

    with tile.TileContext(nc) as tc:
        with (
            tc.tile_pool(name="const", bufs=1) as cpool,
            tc.tile_pool(name="stream", bufs=SBUFS) as spool,
            tc.tile_pool(name="acc", bufs=1, space="PSUM") as apool,
            tc.tile_pool(name="outs", bufs=2) as opool,
        ):
            w_sb = cpool.tile([128, KCH, PL, NW_PAD], sdt, tag="whl")
            nc.sync.dma_start(w_sb[:], whl[:])

            # logits.T accumulators: NQB psum banks of [8, QBLK] fp32, held
            # across the whole contraction.
            accs = [
                apool.tile([NW_PAD, QBLK], f32, tag=f"acc{b}", name=f"acc{b}")
                for b in range(NQB)
            ]

            split = int(os.environ.get("MK_SPLIT_DMA", "1"))
            n_slabs = KCH // SLAB
            for j in range(n_slabs):
                slab = spool.tile([128, SLAB, PL, NQ_PAD], sdt, tag="slab")
                src = xt[ts(j, SLAB)].rearrange("o p l q -> p o l q")
                if split > 1:
                    step = NQ_PAD // split
                    for si in range(split):
                        nc.sync.dma_start(
                            slab[:, :, :, ts(si, step)], src[:, :, :, ts(si, step)]
                        )
                else:
                    nc.sync.dma_start(slab[:], src)
                for o in range(SLAB):
                    k = j * SLAB + o
                    for b in range(NQB):
                        for ci, (xp, wp) in enumerate(combos):
                            nc.tensor.matmul(
                                accs[b][:],
                                w_sb[:, k, wp],
                                slab[:, o, xp, ts(b, QBLK)],
                                start=(k == 0 and ci == 0),
                                stop=(k == KCH - 1 and ci == len(combos) - 1),
                            )

            out_sb = opool.tile([NW_PAD, NQ_PAD], f32, tag="out")
            for b in range(NQB):
                nc.vector.tensor_copy(out_sb[:, ts(b, QBLK)], accs[b][:])
            nc.sync.dma_start(outT[:], out_sb[:])

    nc.compile()
    _BUILD_CACHE[key] = nc
    return nc


# ------------------------------------------------------------ input packing
def _split_hilo(a):
    """float32 array -> (hi, lo) bf16 arrays with a ~= hi + lo."""
    import ml_dtypes

    bf16 = np.dtype(ml_dtypes.bfloat16)
    hi = a.astype(bf16)
    lo = (a - hi.astype(np.float32)).astype(bf16)
    return hi, lo


def _pack_shards(query):
    """query [15000, 4096] f32 -> per-core feature-major [KCH,128,PL,NQ_PAD]."""
    sdt = _np_stream_dtype()
    if STREAM_DT == "hilo":
        planes = _split_hilo(query)
    else:
        planes = (query.astype(sdt, copy=False),)
    shards = []
    for c in range(N_CORES):
        arr = np.zeros((KCH, 128, PL, NQ_PAD), sdt)
        for pl, q in enumerate(planes):
            qs = q[c * NQ_SHARD : (c + 1) * NQ_SHARD]  # [1875, 4096]
            # blocked transpose: per k-chunk, copy [1875, 128] -> [128, 1875]
            for o in range(KCH):
                arr[o, :, pl, :NQ_SHARD] = qs[:, o * 128 : (o + 1) * 128].T
        shards.append(arr)
    return shards


def _pack_w(support, qp):
    """W = sup.T @ qp [4096, 5] f32 -> [128, KCH, PL, NW_PAD] feature-major."""
    W = np.zeros((D, NW_PAD), np.float32)
    W[:, :N_WAY] = support.T @ qp
    sdt = _np_stream_dtype()
    if STREAM_DT == "hilo":
        planes = _split_hilo(W)
    else:
        planes = (W.astype(sdt, copy=False),)
    whl = np.zeros((128, KCH, PL, NW_PAD), sdt)
    for pl, w in enumerate(planes):
        # whl[p, o, pl, c] = w[o*128 + p, c]
        whl[:, :, pl, :] = w.reshape(KCH, 128, NW_PAD).transpose(1, 0, 2)
    return np.ascontiguousarray(whl)


def kernel(x):
    x = np.ascontiguousarray(np.asarray(x, dtype=np.float32))
    xr = x.reshape(N_WAY, N_SUPPORT + N_QUERY, D)
    support = np.ascontiguousarray(xr[:, :N_SUPPORT].reshape(NS, D))
    query = np.ascontiguousarray(xr[:, N_SUPPORT:].reshape(NQ_TOT, D))

    # --- host: tiny QP solve (replicated, mirrors reference numerics)
    K = support @ support.T
    qp = _qp_solve_host(K)                              # [25, 5] f32

    whl = _pack_w(support, qp)
    shards = _pack_shards(query)

    in_maps = [{"xt": shards[c], "whl": whl} for c in range(N_CORES)]

    res = None
    last_err = None
    for attempt in range(3):
        try:
            from concourse.bass_utils import run_bass_kernel_spmd

            nc = _build_bass()
            res = run_bass_kernel_spmd(
                nc, in_maps, core_ids=list(range(N_CORES))
            )
            break
        except Exception as e:  # transient device/compile hiccups
            last_err = e
            import sys, time, traceback

            traceback.print_exc()
            word = "retrying" if attempt < 2 else "giving up"
            print(
                f"kernel: device attempt {attempt} failed "
                f"({type(e).__name__}), {word}",
                file=sys.stderr,
            )
            time.sleep(2.0 * (attempt + 1))

    if res is not None:
        logits = np.empty((NQ_TOT, N_WAY), np.float32)
        for c in range(N_CORES):
            outT = res.results[c]["outT"]               # [NW_PAD, NQ_PAD]
            logits[c * NQ_SHARD : (c + 1) * NQ_SHARD] = (
                outT[:N_WAY, :NQ_SHARD].T
            )
        return logits

    # last-resort host fallback: numerically correct, no device speedup
    import sys

    print(
        f"kernel: falling back to host compute after device failure: "
        f"{last_err!r}",
        file=sys.stderr,
    )
    return ((query @ support.T) @ qp).astype(np.float32)



# revision 2
# speedup vs baseline: 3.0782x; 3.0782x over previous
"""MetaOptNet episode kernel for 8x Trainium2 NeuronCores.

Math (from the reference nn.Module):
    x: [15025, 4096] = 5 classes x (5 support + 3000 query) rows.
    K = support @ support.T  (25x25)
    qp = interior-point solve of a tiny 125-var SVM dual (15 fixed iterations)
    logits = (query @ support.T) @ qp        -> [15000, 5]

Split of work:
  - The QP solve is a tiny serial 125-variable problem; it is replicated on
    the host in float32, exactly mirroring the reference algorithm.
  - The memory-bound bulk (streaming the 245 MB of query rows against
    W = support.T @ qp) runs on the 8 NeuronCores, data-parallel over query
    rows (1875 queries per core).

Device kernel design (per core):
  - The query stream is quantized host-side to fp8 e3m4 (x * 2, exactly
    invertible scale), quartering HBM traffic vs the fp32 baseline. W is
    carried as an e3m4 hi+lo pair (W * 2048), making its quantization error
    negligible; the logits are divided by 4096 on the host afterwards.
    End-to-end relative error ~1.1e-2 (gate: 2e-2), dominated by the x
    quantization, verified deterministically on the fixed episode inputs.
  - Matmuls run x-stationary: lhsT = a [128, 125] feature-major query tile
    (PE stationary array), rhs = the [128, 8] W chunk (moving). Each query
    tile accumulates its 32 k-chunks x 2 planes into a private PSUM bank
    ([125, 8] f32); accumulation groups must not share a PSUM bank, so at
    most 8 tiles are in flight and tiles cycle through 6 banks.
  - Queries are streamed chunk-major (chunks of 3/3/3/3/2/1 tiles of 125).
    A chunk's k-slabs arrive as [128, kslab, csz] fp8 DMAs (>=2KB per
    partition row, full 360 GB/s); its tiles' outputs are copied and stored
    while later chunks stream, so only the tiny last chunk sits in the
    serial tail.
"""

import os

import numpy as np

# ---------------------------------------------------------------- constants
N_WAY = 5
N_SUPPORT = 5
N_QUERY = 3000
D = 4096
C_REG = 0.1
MAX_ITER = 15
SIGMA = 0.1

N_CORES = 8
NS = N_WAY * N_SUPPORT          # 25 support rows
NQ_TOT = N_WAY * N_QUERY        # 15000 query rows
NQ_SHARD = NQ_TOT // N_CORES    # 1875 per core
KCH = D // 128                  # 32 contraction chunks of 128
TSZ = 125                       # query tile rows (PSUM group partition dim)
NT_TOT = NQ_SHARD // TSZ        # 15 tiles per core
NW_PAD = 8                      # classes padded (zero) to 8

SX = 2.0                        # x quantization scale (power of 2, exact)
SW = 2048.0                     # W quantization scale (power of 2, exact)

# chunk layout: tiles per chunk (sum must be NT_TOT); small chunk last so
# the serial tail after the stream is minimal
CHUNK_TILES = tuple(
    int(t) for t in os.environ.get("MK_CHUNKS", "3,3,3,3,2,1").split(",")
)
KSLAB = int(os.environ.get("MK_KSLAB", "16"))   # k-chunks per stream DMA
N_BANKS = int(os.environ.get("MK_BANKS", "6"))  # PSUM banks cycled by tiles
assert sum(CHUNK_TILES) == NT_TOT and KCH % KSLAB == 0


def _chunk_starts():
    starts, t0 = [], 0
    for nt in CHUNK_TILES:
        starts.append(t0)
        t0 += nt
    return starts


# ------------------------------------------------------------ host QP solve
def _qp_solve_host(K):
    """Mirror of reference._qp_solve for this problem's fixed G/e/C/h/A/b.

    C is the identity and b is zero, so C-products are elided (exact in
    fp32).  All arithmetic in float32 to track the reference's rounding.
    """
    dt = np.float32
    n = NS * N_WAY                                    # 125
    m, p = n, NS                                      # 125, 25
    G = np.kron(K, np.eye(N_WAY, dtype=dt)).astype(dt) + np.eye(n, dtype=dt)
    y = np.repeat(np.arange(N_WAY), N_SUPPORT)
    y1 = np.eye(N_WAY, dtype=dt)[y].reshape(-1)       # [125] one-hot flat
    e = -y1
    h = (dt(C_REG) * y1).astype(dt)
    A = np.kron(np.eye(NS, dtype=dt), np.ones((1, N_WAY), dtype=dt)).astype(dt)
    sigma = dt(SIGMA)

    z = np.zeros(n, dt)
    s = np.ones(m, dt)
    lam = np.ones(m, dt)
    nu = np.zeros(p, dt)

    for _ in range(MAX_ITER):
        r_dual = G @ z + e + lam + A.T @ nu
        r_pin = z + s - h
        r_peq = A @ z
        mu = np.dot(s, lam) / dt(m)
        r_cent = s * lam - sigma * mu
        w = lam / s
        M = G + np.diag(w).astype(dt)
        rhs_z = -(r_dual + (-r_cent + lam * r_pin) / s)
        KKT = np.block([[M, A.T], [A, np.zeros((p, p), dt)]]).astype(dt)
        sol = np.linalg.solve(KKT, np.concatenate([rhs_z, -r_peq]))
        dz, dnu = sol[:n], sol[n:]
        ds = -r_pin - dz
        dlam = (-r_cent - lam * ds) / s
        with np.errstate(divide="ignore", invalid="ignore"):
            a_s = np.min(np.where(ds < 0, -s / ds, np.inf)).astype(dt)
            a_l = np.min(np.where(dlam < 0, -lam / dlam, np.inf)).astype(dt)
        alpha = np.minimum(dt(1.0), dt(0.99) * np.minimum(a_s, a_l))
        z = z + alpha * dz
        s = s + alpha * ds
        lam = lam + alpha * dlam
        nu = nu + alpha * dnu

    return z.reshape(NS, N_WAY)                       # [25, 5]


# ------------------------------------------------------------- bass builder
_BUILD_CACHE = {}


def _np_f8():
    import ml_dtypes

    return np.dtype(ml_dtypes.float8_e3m4)


def _build_bass():
    key = (CHUNK_TILES, KSLAB, N_BANKS)
    if key in _BUILD_CACHE:
        return _BUILD_CACHE[key]

    import concourse.bacc as bacc
    import concourse.mybir as mybir
    import concourse.tile as tile

    f8 = mybir.dt.float8e3
    f32 = mybir.dt.float32

    nc = bacc.Bacc("TRN2", target_bir_lowering=False, debug=False)
    xts = [
        nc.dram_tensor(f"xt{g}", [128, KCH, nt * TSZ], f8, kind="ExternalInput")
        for g, nt in enumerate(CHUNK_TILES)
    ]
    # W hi/lo planes, feature-major: whl[p, k, pl, c] = Wplane[k*128+p, c]
    whl = nc.dram_tensor("whl", [128, KCH, 2, NW_PAD], f8, kind="ExternalInput")
    outT = nc.dram_tensor("outT", [TSZ, NT_TOT, NW_PAD], f32, kind="ExternalOutput")

    starts = _chunk_starts()

    with tile.TileContext(nc) as tc:
        with (
            tc.tile_pool(name="const", bufs=1) as cpool,
            tc.tile_pool(name="stream", bufs=1) as spool,
            tc.tile_pool(name="acc", bufs=1, space="PSUM") as apool,
        ):
            w_sb = cpool.tile([128, KCH, 2, NW_PAD], f8, tag="whl")
            nc.sync.dma_start(w_sb[:], whl[:])

            # one PSUM bank per in-flight query tile; tile i -> bank i % N_BANKS
            accs = [
                apool.tile([128, NW_PAD], f32, tag=f"acc{s}", name=f"acc{s}")
                for s in range(N_BANKS)
            ]
            out_sb = cpool.tile([128, NT_TOT, NW_PAD], f32, tag="out")

            for g, nt in enumerate(CHUNK_TILES):
                csz = nt * TSZ
                t0 = starts[g]
                for k0 in range(0, KCH, KSLAB):
                    slab = spool.tile(
                        [128, KSLAB, csz], f8,
                        tag=f"slab{g}_{k0}", name=f"slab{g}_{k0}",
                    )
                    nc.sync.dma_start(slab[:], xts[g][:, k0 : k0 + KSLAB, :])
                    for tl in range(nt):
                        acc = accs[(t0 + tl) % N_BANKS]
                        for kk in range(KSLAB):
                            k = k0 + kk
                            for pl in range(2):
                                nc.tensor.matmul(
                                    acc[:TSZ, :],
                                    slab[:, kk, tl * TSZ : (tl + 1) * TSZ],
                                    w_sb[:, k, pl, :],
                                    start=(k == 0 and pl == 0),
                                    stop=(k == KCH - 1 and pl == 1),
                                )
                # chunk done: drain its PSUM banks and store its logits
                for tl in range(nt):
                    nc.vector.tensor_copy(
                        out_sb[:TSZ, t0 + tl, :], accs[(t0 + tl) % N_BANKS][:TSZ, :]
                    )
                nc.sync.dma_start(
                    outT[:, t0 : t0 + nt, :], out_sb[:TSZ, t0 : t0 + nt, :]
                )

    nc.compile()
    _BUILD_CACHE[key] = nc
    return nc


# ------------------------------------------------------------ input packing
def _pack_shards(query):
    """query [15000, 4096] f32 -> per-core dict of chunk tensors + planes."""
    f8 = _np_f8()
    xq = (query * np.float32(SX)).astype(f8)          # [15000, 4096] e3m4
    starts = _chunk_starts()
    shards = []
    for c in range(N_CORES):
        qs = xq[c * NQ_SHARD : (c + 1) * NQ_SHARD]    # [1875, 4096]
        chunk_map = {}
        for g, nt in enumerate(CHUNK_TILES):
            csz = nt * TSZ
            q0 = starts[g] * TSZ
            blk = qs[q0 : q0 + csz]                   # [csz, 4096]
            # [csz, KCH, 128] -> [128, KCH, csz]
            chunk_map[f"xt{g}"] = np.ascontiguousarray(
                blk.reshape(csz, KCH, 128).transpose(2, 1, 0)
            )
        shards.append(chunk_map)
    return shards


def _pack_w(support, qp):
    """W = sup.T @ qp [4096, 5] -> e3m4 hi/lo planes [128, KCH, 2, NW_PAD]."""
    f8 = _np_f8()
    W = np.zeros((D, NW_PAD), np.float32)
    W[:, :N_WAY] = support.T @ qp
    Wt = W * np.float32(SW)
    whi = Wt.astype(f8)
    wlo = (Wt - whi.astype(np.float32)).astype(f8)
    whl = np.zeros((128, KCH, 2, NW_PAD), f8)
    for pl, w in enumerate((whi, wlo)):
        whl[:, :, pl, :] = w.reshape(KCH, 128, NW_PAD).transpose(1, 0, 2)
    return np.ascontiguousarray(whl)


def kernel(x):
    x = np.ascontiguousarray(np.asarray(x, dtype=np.float32))
    xr = x.reshape(N_WAY, N_SUPPORT + N_QUERY, D)
    support = np.ascontiguousarray(xr[:, :N_SUPPORT].reshape(NS, D))
    query = np.ascontiguousarray(xr[:, N_SUPPORT:].reshape(NQ_TOT, D))

    # --- host: tiny QP solve (replicated, mirrors reference numerics)
    K = support @ support.T
    qp = _qp_solve_host(K)                              # [25, 5] f32

    whl = _pack_w(support, qp)
    shards = _pack_shards(query)

    in_maps = [dict(shards[c], whl=whl) for c in range(N_CORES)]

    res = None
    last_err = None
    for attempt in range(3):
        try:
            from concourse.bass_utils import run_bass_kernel_spmd

            nc = _build_bass()
            res = run_bass_kernel_spmd(
                nc, in_maps, core_ids=list(range(N_CORES))
            )
            break
        except Exception as e:  # transient device/compile hiccups
            last_err = e
            import sys, time, traceback

            traceback.print_exc()
            word = "retrying" if attempt < 2 else "giving up"
            print(
                f"kernel: device attempt {attempt} failed "
                f"({type(e).__name__}), {word}",
                file=sys.stderr,
            )
            time.sleep(2.0 * (attempt + 1))

    inv = np.float32(1.0 / (SX * SW))
    if res is not None:
        logits = np.empty((NQ_TOT, N_WAY), np.float32)
        for c in range(N_CORES):
            outT = res.results[c]["outT"]               # [125, 15, 8]
            logits[c * NQ_SHARD : (c + 1) * NQ_SHARD] = (
                outT.transpose(1, 0, 2).reshape(NQ_SHARD, NW_PAD)[:, :N_WAY]
                * inv
            )
        return logits

    # last-resort host fallback: numerically correct, no device speedup
    import sys

    print(
        f"kernel: falling back to host compute after device failure: "
        f"{last_err!r}",
        file=sys.stderr,
    )
    return ((query @ support.T) @ qp).astype(np.float32)


# revision 16
# speedup vs baseline: 3.4300x; 1.1143x over previous
"""MetaOptNet episode kernel for 8x Trainium2 NeuronCores.

Math (from the reference nn.Module):
    x: [15025, 4096] = 5 classes x (5 support + 3000 query) rows.
    K = support @ support.T  (25x25)
    qp = interior-point solve of a tiny 125-var SVM dual (15 fixed iterations)
    logits = (query @ support.T) @ qp        -> [15000, 5]

Split of work:
  - The QP solve is a tiny serial 125-variable problem; it is replicated on
    the host in float32, exactly mirroring the reference algorithm.
  - The memory-bound bulk (streaming the 245 MB of query rows against
    W = support.T @ qp) runs on the 8 NeuronCores, data-parallel over query
    rows (1875 queries per core).

Device kernel design (per core):
  - The query stream is quantized host-side to fp8 e3m4 (x * 2, exactly
    invertible scale), quartering HBM traffic vs the fp32 baseline. W is
    carried as an e3m4 hi+lo pair (W * 2048), making its quantization error
    negligible; the logits are divided by 4096 on the host afterwards.
    End-to-end relative error ~1.1e-2 (gate: 2e-2), dominated by the x
    quantization, verified deterministically on the fixed episode inputs.
  - Matmuls run x-stationary: lhsT = a [128, 125] feature-major query tile
    (PE stationary array), rhs = the [128, 8] W chunk (moving). Each query
    tile accumulates its 32 k-chunks x 2 planes into a private PSUM bank
    ([125, 8] f32); accumulation groups must not share a PSUM bank, so at
    most 8 tiles are in flight and tiles cycle through 6 banks.
  - Queries are streamed chunk-major (chunks of 3/3/3/3/2/1 tiles of 125).
    A chunk's k-slabs arrive as [128, kslab, csz] fp8 DMAs (>=2KB per
    partition row, full 360 GB/s); its tiles' outputs are copied and stored
    while later chunks stream, so only the tiny last chunk sits in the
    serial tail.
"""

import os

import numpy as np

# ---------------------------------------------------------------- constants
N_WAY = 5
N_SUPPORT = 5
N_QUERY = 3000
D = 4096
C_REG = 0.1
MAX_ITER = 15
SIGMA = 0.1

N_CORES = 8
NS = N_WAY * N_SUPPORT          # 25 support rows
NQ_TOT = N_WAY * N_QUERY        # 15000 query rows
NQ_SHARD = NQ_TOT // N_CORES    # 1875 per core
KCH = D // 128                  # 32 contraction chunks of 128
TSZ = 125                       # query tile rows (PSUM group partition dim)
NT_TOT = NQ_SHARD // TSZ        # 15 tiles per core
NW_PAD = 8                      # classes padded (zero) to 8

SX = 2.0                        # x quantization scale (power of 2, exact)
SW_TARGET = 15.0                # per-column W scale target absmax (e3m4 max 15.5)

# chunk layout: "tiles:kslab+kslab+...," per chunk (tiles sum to NT_TOT);
# small chunk last (with a tiny final k-slab) so the serial tail after the
# stream is minimal
_CHUNK_SPEC = os.environ.get(
    "MK_CHUNKS", "3:32,3:32,3:32,3:32,2:32,1:28+4"
)
CHUNKS = tuple(
    (int(part.split(":")[0]),
     tuple(int(s) for s in part.split(":")[1].split("+")))
    for part in _CHUNK_SPEC.split(",")
)
CHUNK_TILES = tuple(nt for nt, _ in CHUNKS)
N_BANKS = int(os.environ.get("MK_BANKS", "6"))  # PSUM banks cycled by tiles
assert sum(CHUNK_TILES) == NT_TOT
assert all(sum(ks) == KCH for _, ks in CHUNKS)


def _chunk_starts():
    starts, t0 = [], 0
    for nt in CHUNK_TILES:
        starts.append(t0)
        t0 += nt
    return starts


# ------------------------------------------------------------ host QP solve
def _qp_solve_host(K):
    """Mirror of reference._qp_solve for this problem's fixed G/e/C/h/A/b.

    C is the identity and b is zero, so C-products are elided (exact in
    fp32).  All arithmetic in float32 to track the reference's rounding.
    """
    dt = np.float32
    n = NS * N_WAY                                    # 125
    m, p = n, NS                                      # 125, 25
    G = np.kron(K, np.eye(N_WAY, dtype=dt)).astype(dt) + np.eye(n, dtype=dt)
    y = np.repeat(np.arange(N_WAY), N_SUPPORT)
    y1 = np.eye(N_WAY, dtype=dt)[y].reshape(-1)       # [125] one-hot flat
    e = -y1
    h = (dt(C_REG) * y1).astype(dt)
    A = np.kron(np.eye(NS, dtype=dt), np.ones((1, N_WAY), dtype=dt)).astype(dt)
    sigma = dt(SIGMA)

    z = np.zeros(n, dt)
    s = np.ones(m, dt)
    lam = np.ones(m, dt)
    nu = np.zeros(p, dt)

    for _ in range(MAX_ITER):
        r_dual = G @ z + e + lam + A.T @ nu
        r_pin = z + s - h
        r_peq = A @ z
        mu = np.dot(s, lam) / dt(m)
        r_cent = s * lam - sigma * mu
        w = lam / s
        M = G + np.diag(w).astype(dt)
        rhs_z = -(r_dual + (-r_cent + lam * r_pin) / s)
        KKT = np.block([[M, A.T], [A, np.zeros((p, p), dt)]]).astype(dt)
        sol = np.linalg.solve(KKT, np.concatenate([rhs_z, -r_peq]))
        dz, dnu = sol[:n], sol[n:]
        ds = -r_pin - dz
        dlam = (-r_cent - lam * ds) / s
        with np.errstate(divide="ignore", invalid="ignore"):
            a_s = np.min(np.where(ds < 0, -s / ds, np.inf)).astype(dt)
            a_l = np.min(np.where(dlam < 0, -lam / dlam, np.inf)).astype(dt)
        alpha = np.minimum(dt(1.0), dt(0.99) * np.minimum(a_s, a_l))
        z = z + alpha * dz
        s = s + alpha * ds
        lam = lam + alpha * dlam
        nu = nu + alpha * dnu

    return z.reshape(NS, N_WAY)                       # [25, 5]


# ------------------------------------------------------------- bass builder
_BUILD_CACHE = {}


def _np_f8():
    import ml_dtypes

    return np.dtype(ml_dtypes.float8_e3m4)


def _build_bass():
    key = (CHUNKS, N_BANKS)
    if key in _BUILD_CACHE:
        return _BUILD_CACHE[key]

    import concourse.bacc as bacc
    import concourse.mybir as mybir
    import concourse.tile as tile

    f8 = mybir.dt.float8e3
    f32 = mybir.dt.float32

    nc = bacc.Bacc("TRN2", target_bir_lowering=False, debug=False)
    # chunk 0 carries W appended per k-chunk: [csz stream | 2*NW_PAD W bytes]
    # so no separate W DMA is needed (8-HWDGE-semaphore budget: reusing a
    # semaphore makes a later DMA wait on an unrelated earlier one)
    xts = [
        nc.dram_tensor(
            f"xt{g}",
            [128, KCH, nt * TSZ + (2 * NW_PAD if g == 0 else 0)],
            f8,
            kind="ExternalInput",
        )
        for g, nt in enumerate(CHUNK_TILES)
    ]
    outT = nc.dram_tensor("outT", [TSZ, NT_TOT, NW_PAD], f32, kind="ExternalOutput")

    starts = _chunk_starts()

    with tile.TileContext(nc) as tc:
        with (
            tc.tile_pool(name="const", bufs=1) as cpool,
            tc.tile_pool(name="stream", bufs=1) as spool,
            tc.tile_pool(name="acc", bufs=1, space="PSUM") as apool,
        ):
            slabs = {}
            for g, (nt, kslabs) in enumerate(CHUNKS):
                csz = nt * TSZ + (2 * NW_PAD if g == 0 else 0)
                k0 = 0
                for ks in kslabs:
                    slab = spool.tile(
                        [128, ks, csz], f8,
                        tag=f"slab{g}_{k0}", name=f"slab{g}_{k0}",
                    )
                    nc.sync.dma_start(slab[:], xts[g][:, k0 : k0 + ks, :])
                    slabs[g, k0] = (slab, ks)
                    k0 += ks
            # W slices live inside chunk0's slabs: w_at(k) -> [128, 2*NW_PAD]
            c0 = CHUNK_TILES[0] * TSZ
            k0s_0 = []
            k0 = 0
            for ks in CHUNKS[0][1]:
                k0s_0.append((k0, ks))
                k0 += ks

            def w_at(k, pl):
                for kk0, ks in k0s_0:
                    if kk0 <= k < kk0 + ks:
                        return slabs[0, kk0][0][
                            :, k - kk0, c0 + pl * NW_PAD : c0 + (pl + 1) * NW_PAD
                        ]
                raise AssertionError(k)

            # one PSUM bank per in-flight query tile; tile i -> bank i % N_BANKS
            accs = [
                apool.tile([128, NW_PAD], f32, tag=f"acc{s}", name=f"acc{s}")
                for s in range(N_BANKS)
            ]
            out_sb = cpool.tile([128, NT_TOT, NW_PAD], f32, tag="out")

            for g, (nt, kslabs) in enumerate(CHUNKS):
                t0 = starts[g]
                k0 = 0
                for ks in kslabs:
                    slab, _ = slabs[g, k0]
                    for tl in range(nt):
                        acc = accs[(t0 + tl) % N_BANKS]
                        for kk in range(ks):
                            k = k0 + kk
                            for pl in range(2):
                                nc.tensor.matmul(
                                    acc[:TSZ, :],
                                    slab[:, kk, tl * TSZ : (tl + 1) * TSZ],
                                    w_at(k, pl),
                                    start=(k == 0 and pl == 0),
                                    stop=(k == KCH - 1 and pl == 1),
                                )
                    k0 += ks
                # chunk done: drain its PSUM banks and store its logits.
                # Early outs ride the Pool SWDGE queue (own semaphore space,
                # desc-gen on the otherwise idle Pool engine) so the stream's
                # HWDGE semaphores are never entangled with out completions;
                # the final out uses SP (streams all issued by then) for the
                # shorter HWDGE + DGE->DMA path in the serial tail.
                for tl in range(nt):
                    nc.vector.tensor_copy(
                        out_sb[:TSZ, t0 + tl, :], accs[(t0 + tl) % N_BANKS][:TSZ, :]
                    )
                out_eng = nc.gpsimd if g < len(CHUNKS) - 1 else nc.sync
                out_eng.dma_start(
                    outT[:, t0 : t0 + nt, :], out_sb[:TSZ, t0 : t0 + nt, :]
                )

    nc.compile()
    _BUILD_CACHE[key] = nc
    return nc


# ------------------------------------------------------------ input packing
def _pack_shards(query, whl):
    """query [15000, 4096] f32 -> per-core dict of chunk tensors.

    whl [128, KCH, 2, NW_PAD] e3m4 W planes are appended to chunk 0's
    per-k columns so the whole episode needs no separate W DMA.
    """
    f8 = _np_f8()
    xq = (query * np.float32(SX)).astype(f8)          # [15000, 4096] e3m4
    wcols = whl.reshape(128, KCH, 2 * NW_PAD)
    starts = _chunk_starts()
    shards = []
    for c in range(N_CORES):
        qs = xq[c * NQ_SHARD : (c + 1) * NQ_SHARD]    # [1875, 4096]
        chunk_map = {}
        for g, nt in enumerate(CHUNK_TILES):
            csz = nt * TSZ
            q0 = starts[g] * TSZ
            blk = qs[q0 : q0 + csz]                   # [csz, 4096]
            # [csz, KCH, 128] -> [128, KCH, csz]
            arr = blk.reshape(csz, KCH, 128).transpose(2, 1, 0)
            if g == 0:
                arr = np.concatenate([arr, wcols], axis=2)
            chunk_map[f"xt{g}"] = np.ascontiguousarray(arr)
        shards.append(chunk_map)
    return shards


def _pack_w(support, qp):
    """W = sup.T @ qp [4096, 5] -> e3m4 hi/lo planes [128, KCH, 2, NW_PAD].

    Each class column gets its own power-of-2 scale pushing it to the top of
    e3m4's normal range, so the lo plane's residual (denormal floor) is as
    small as possible relative to the column. Returns (whl, col_scales).
    """
    f8 = _np_f8()
    W = np.zeros((D, NW_PAD), np.float32)
    W[:, :N_WAY] = support.T @ qp
    absmax = np.abs(W).max(axis=0)
    scales = np.where(
        absmax > 0,
        np.exp2(np.floor(np.log2(SW_TARGET / np.maximum(absmax, 1e-30)))),
        1.0,
    ).astype(np.float32)
    Wt = W * scales[None, :]
    whi = Wt.astype(f8)
    wlo = (Wt - whi.astype(np.float32)).astype(f8)
    whl = np.zeros((128, KCH, 2, NW_PAD), f8)
    for pl, w in enumerate((whi, wlo)):
        whl[:, :, pl, :] = w.reshape(KCH, 128, NW_PAD).transpose(1, 0, 2)
    return np.ascontiguousarray(whl), scales


def kernel(x):
    x = np.ascontiguousarray(np.asarray(x, dtype=np.float32))
    xr = x.reshape(N_WAY, N_SUPPORT + N_QUERY, D)
    support = np.ascontiguousarray(xr[:, :N_SUPPORT].reshape(NS, D))
    query = np.ascontiguousarray(xr[:, N_SUPPORT:].reshape(NQ_TOT, D))

    # --- host: tiny QP solve (replicated, mirrors reference numerics)
    K = support @ support.T
    qp = _qp_solve_host(K)                              # [25, 5] f32

    whl, col_scales = _pack_w(support, qp)
    shards = _pack_shards(query, whl)

    in_maps = [dict(shards[c]) for c in range(N_CORES)]

    res = None
    last_err = None
    for attempt in range(3):
        try:
            from concourse.bass_utils import run_bass_kernel_spmd

            nc = _build_bass()
            res = run_bass_kernel_spmd(
                nc, in_maps, core_ids=list(range(N_CORES))
            )
            break
        except Exception as e:  # transient device/compile hiccups
            last_err = e
            import sys, time, traceback

            traceback.print_exc()
            word = "retrying" if attempt < 2 else "giving up"
            print(
                f"kernel: device attempt {attempt} failed "
                f"({type(e).__name__}), {word}",
                file=sys.stderr,
            )
            time.sleep(2.0 * (attempt + 1))

    inv = (1.0 / (SX * col_scales[:N_WAY])).astype(np.float32)
    if res is not None:
        logits = np.empty((NQ_TOT, N_WAY), np.float32)
        for c in range(N_CORES):
            outT = res.results[c]["outT"]               # [125, 15, 8]
            logits[c * NQ_SHARD : (c + 1) * NQ_SHARD] = (
                outT.transpose(1, 0, 2).reshape(NQ_SHARD, NW_PAD)[:, :N_WAY]
                * inv[None, :]
            )
        return logits

    # last-resort host fallback: numerically correct, no device speedup
    import sys

    print(
        f"kernel: falling back to host compute after device failure: "
        f"{last_err!r}",
        file=sys.stderr,
    )
    return ((query @ support.T) @ qp).astype(np.float32)
